# revision 23
# baseline (speedup 1.0000x reference)
"""Trainium2 Bass kernel for nn_Capsule_16484084482446.

Reference math collapses: with cw = softmax(rw, axis=1),
  outputs[b,j,d] = sum_i sum_n cw[b,i,n] * u[b,j,n,d]
                 = sum_n u[b,j,n,d]           (since sum_i cw[b,i,n] == 1)
so the routing loop is a no-op and the final result is
  out = (sum_n x[b,n,:]) @ W   reshaped to (B, 10, 16).

Pure HBM-read problem (64 MB of x). x and W are uploaded as fp16
(host-side cast): halves HBM traffic and DVE fold work; measured
rel-err ~6e-4 vs the 2e-2 gate.

Per core (4 batches x 8 cores, data-parallel over batch):
  x_shard (4, 4096, 128) fp16 viewed as 128 partitions x 128 rows x 128 d;
  partition p holds rows [128p, 128p+128), batch b owns partitions
  [32b, 32b+32).
  1. Few big chunked HWDGE DMAs: 32-row chunks first (their 2.1 us
     folds must finish before the stream ends), only no-fold 8/4-row
     chunks in the straggle-compressed final ~1 us. Two mid-stream
     16-row chunks ride the scalar ring, shortening the sync ring's
     queue (the one-laggard-SDMA-engine straggle grows with ring
     depth). The scalar ring also carries a 45 KB constants tensor
     first: a [128, 16] 0/1 mask (columns = 16-partition half-slabs of
     each batch) concatenated with W, in fp16.
  2. VectorE prefolds 16/32-row chunks in place down to 8 rows with
     contiguous halving adds (~0.8 us per 0.5 MB -- comfortably under
     the ~1.2 us/0.5 MB arrival rate, unlike fold-to-4 which stalls).
     8/4-row chunks skip DVE entirely (PE takes them off their DMA sem).
  3. PE accumulates 256-col slab-pairs into psum_s[16, 256] with the
     mask as stationary (16-col LDWEIGHTS ~95ns, ~250ns per matmul;
     4 matmuls per folded chunk). psum col n2*128+d = batch-half sum
     over rows of parity n2.
  4. Tail: DVE cast-copies psum_s to fp16, one 128-col fold, 4 32x32
     block transposes to (128,16), two halving adds over the half-slab
     columns -> s^T (128,4), PE does the final s^T @ W matmul in fp16,
     DVE copies psum_o out, sync DMAs the 640 B result with no
     completion wait (it drains during the NEFF postamble).

Raw Bass (no TileContext); every semaphore is cleared by its final
consumer right after its last wait, so the NEFF re-executes cleanly
(profilers loop it).
"""

from contextlib import ExitStack

import numpy as np

import concourse.bass as bass
from concourse import mybir
from concourse.bass_utils import run_bass_kernel_spmd

N_CORES = 8
B, N, DIN = 32, 4096, 128
BSH = B // N_CORES          # 4 batches per core
DOUT = 160                  # 10 capsules * 16 dims
NMASK = 16                  # mask columns: 4 batches x 4 16-partition slabs
NF = 256                    # matmul moving width: one 2-row slab-pair
# rows-per-partition split; 16/32-row chunks halve down to 8 rows on
# DVE, 8/4-row chunks go straight to PE. Tiny tail chunks keep the
# final matmuls cheap.
# 32-row chunks go first so their 2.1us folds finish well before the
# stream ends; only no-fold 4-row chunks arrive in the final ~1us
CHUNKS = [32, 32, 16, 16, 16, 8, 4, 4]
# mid-stream 16-row chunks ride the scalar ring: shortens the sync
# ring's queue (the one-laggard-engine straggle grows with ring depth)
# without putting any late, tail-gating chunk on the slow-ramping
# scalar ring
SCALAR_CHUNKS = (2, 3)
PREFOLD_ROWS = 8
assert sum(CHUNKS) == BSH * N // 128
NCHUNK = len(CHUNKS)

F16 = mybir.dt.float16
F32 = mybir.dt.float32

_cache = {}


def _build_nc(chunks=None):
    global CHUNKS, NCHUNK
    if chunks is not None:
        CHUNKS = chunks
        NCHUNK = len(CHUNKS)
    assert sum(CHUNKS) == BSH * N // 128
    folded = []
    for c in CHUNKS:
        s = c
        while s > PREFOLD_ROWS:
            assert s % 2 == 0
            s //= 2
        folded.append(c > PREFOLD_ROWS)
        assert (s * DIN) % NF == 0

    nc = bass.Bass()
    x = nc.dram_tensor("x", [BSH, N, DIN], F16, kind="ExternalInput")
    # wm = [mask16 | W]: [128, 16 + 160] fp16 constants, one DMA
    wm = nc.dram_tensor("WM", [DIN, NMASK + DOUT], F16, kind="ExternalInput")
    out = nc.dram_tensor("out", [BSH, DOUT], F32, kind="ExternalOutput")

    # (128, 128, 128): partition p, row-in-partition n, feature d
    x3 = x[:].flatten_outer_dims().rearrange("(p n) d -> p n d", p=128)
    starts = np.cumsum([0] + CHUNKS).tolist()

    with ExitStack() as ctx:
        ec = ctx.enter_context
        xc = [ec(nc.sbuf_tensor(f"xc{c}", [128, CHUNKS[c] * DIN], F16))
              for c in range(NCHUNK)]
        wm_sb = ec(nc.sbuf_tensor("wm_sb", [DIN, NMASK + DOUT], F16))
        s_sb = ec(nc.sbuf_tensor("s_sb", [32, NF], F16))     # rows 0-15 valid
        st_sb = ec(nc.sbuf_tensor("st_sb", [DIN, 32], F16))  # cols 0-15 valid
        out_sb = ec(nc.sbuf_tensor("out_sb", [BSH, DOUT], F32))
        psum_s = ec(nc.psum_tensor("psum_s", [NMASK, NF], F32))
        psum_o = ec(nc.psum_tensor("psum_o", [BSH, DOUT], F32))

        dma_w = ec(nc.semaphore("dma_w"))
        dma_c = [ec(nc.semaphore(f"dma_c{c}")) for c in range(NCHUNK)]
        v_red = ec(nc.semaphore("v_red"))    # +1 per prefolded chunk
        pe_sem = ec(nc.semaphore("pe_sem"))  # +1 psum_s done, +1 psum_o done
        v_t = ec(nc.semaphore("v_t"))        # s^T ready
        v_out = ec(nc.semaphore("v_out"))
        dma_out = ec(nc.semaphore("dma_out"))  # required sync info; unread
        block = ec(nc.Block())

        @block.sync
        def _(sync):
            for c in range(NCHUNK):
                if c in SCALAR_CHUNKS:
                    continue
                sync.dma_start(
                    xc[c][:], x3[:, starts[c] : starts[c + 1], :]
                ).then_inc(dma_c[c], 16)
            sync.wait_ge(v_out, 1)
            sync.sem_clear(v_out)
            # no completion wait: the 640 B store drains during the NEFF
            # postamble; nothing later in this execution reads out_sb.
            # HWDGE requires sync info, so the increment stays, unwaited
            # (dma_out is never read, so its residue is harmless).
            sync.dma_start(out[:], out_sb[:]).then_inc(dma_out, 16)

        @block.scalar
        def _(scalar):
            # second HWDGE ring: constants land early, off the x ring
            scalar.dma_start(wm_sb[:], wm[:]).then_inc(dma_w, 16)
            for c in SCALAR_CHUNKS:
                scalar.dma_start(
                    xc[c][:], x3[:, starts[c] : starts[c + 1], :]
                ).then_inc(dma_c[c], 16)

        @block.vector
        def _(vector):
            for c in range(NCHUNK):
                if not folded[c]:
                    continue          # PE consumes 4/8-row chunks directly
                vector.wait_ge(dma_c[c], 16)
                vector.sem_clear(dma_c[c])
                t = xc[c]
                s = CHUNKS[c]
                while s > PREFOLD_ROWS:
                    s //= 2
                    op = vector.tensor_add(
                        t[:, : s * DIN],
                        t[:, : s * DIN],
                        t[:, s * DIN : 2 * s * DIN],
                    )
                op.then_inc(v_red, 1)
            # psum_s closed by PE -> cast-copy to fp16 SBUF, fold, transpose
            vector.wait_ge(pe_sem, 1)
            vector.tensor_copy(s_sb[0:NMASK, :], psum_s[:])
            vector.tensor_add(
                s_sb[0:NMASK, :DIN],
                s_sb[0:NMASK, :DIN],
                s_sb[0:NMASK, DIN : 2 * DIN],
            )
            # (16,128) -> (128,16) via 32x32 block transposes; rows 16-31
            # of s_sb are junk and land in unused cols 16-31 of st_sb
            for k in range(4):
                vector.transpose(
                    st_sb[32 * k : 32 * (k + 1), 0:32],
                    s_sb[0:32, 32 * k : 32 * (k + 1)],
                )
            # sum the 4 half-slab columns: st[:, h*4+b] over h -> st[:, 0:4]
            vector.tensor_add(st_sb[:, 0:8], st_sb[:, 0:8], st_sb[:, 8:16])
            vector.tensor_add(
                st_sb[:, 0:4], st_sb[:, 0:4], st_sb[:, 4:8]
            ).then_inc(v_t, 1)
            vector.wait_ge(pe_sem, 2)
            vector.sem_clear(pe_sem)
            vector.tensor_copy(out_sb[:], psum_o[:]).then_inc(v_out, 1)

        @block.tensor
        def _(tensor):
            tensor.wait_ge(dma_w, 16)
            tensor.sem_clear(dma_w)
            # psum_s[j, f] += sum_p mask16[p, j] * xc[p, 256k + f]
            nred = 0
            first = True
            last_c = NCHUNK - 1
            for c in range(NCHUNK):
                if folded[c]:
                    nred += 1
                    tensor.wait_ge(v_red, nred)
                    width = PREFOLD_ROWS * DIN
                else:
                    tensor.wait_ge(dma_c[c], 16)
                    tensor.sem_clear(dma_c[c])
                    width = CHUNKS[c] * DIN
                nmm = width // NF
                for k in range(nmm):
                    mm = tensor.matmul(
                        psum_s[:],
                        wm_sb[:, 0:NMASK],
                        xc[c][:, k * NF : (k + 1) * NF],
                        start=first,
                        stop=(c == last_c and k == nmm - 1),
                    )
                    first = False
            # riding the inc on the matmul beats drain().then_inc
            # (measured ~0.85 vs ~1.4 us to sem visibility)
            mm.then_inc(pe_sem, 1)
            tensor.sem_clear(v_red)
            tensor.wait_ge(v_t, 1)
            tensor.sem_clear(v_t)
            # out[b, jd] = sum_d s^T[d, b] * W[d, jd]
            tensor.matmul(
                psum_o[:],
                st_sb[:, 0:BSH],
                wm_sb[:, NMASK : NMASK + DOUT],
                start=True,
                stop=True,
            ).then_inc(pe_sem, 1)

    return nc


def _get_nc():
    if "nc" not in _cache:
        _cache["nc"] = _build_nc()
    return _cache["nc"]


def _make_wm(W):
    # mask16[p, h*4 + b] = 1 iff p in [32b + 16h, 32b + 16h + 16)
    p = np.arange(DIN)
    b = p // 32
    h = (p % 32) // 16
    mask = np.zeros((DIN, NMASK), dtype=np.float16)
    mask[p, h * 4 + b] = 1.0
    return np.concatenate([mask, W.astype(np.float16)], axis=1)


def _in_maps(x, W):
    x = np.ascontiguousarray(x, dtype=np.float16)
    wm = np.ascontiguousarray(_make_wm(np.asarray(W)))
    return [{"x": x[i * BSH : (i + 1) * BSH], "WM": wm} for i in range(N_CORES)]


def kernel(x, W, **profile_kwargs):
    nc = _get_nc()
    res = run_bass_kernel_spmd(nc, _in_maps(x, W), list(range(N_CORES)), **profile_kwargs)
    out = np.concatenate([r["out"] for r in res.results], axis=0)
    ret = out.reshape(B, 10, 16).astype(np.float32)
    if profile_kwargs:
        ret = (ret, res)
    return ret


# revision 27
# speedup vs baseline: 1.1169x; 1.1169x over previous
"""Trainium2 Bass kernel for nn_Capsule_16484084482446.

Reference math collapses: with cw = softmax(rw, axis=1),
  outputs[b,j,d] = sum_i sum_n cw[b,i,n] * u[b,j,n,d]
                 = sum_n u[b,j,n,d]           (since sum_i cw[b,i,n] == 1)
so the routing loop is a no-op and the final result is
  out = (sum_n x[b,n,:]) @ W   reshaped to (B, 10, 16).

Pure HBM-read problem (64 MB of x). x and W are uploaded as fp16
(host-side cast): halves HBM traffic and DVE fold work; measured
rel-err ~6e-4 vs the 2e-2 gate.

Per core (4 batches x 8 cores, data-parallel over batch):
  x_shard (4, 4096, 128) fp16 viewed as 128 partitions x 128 rows x 128 d;
  partition p holds rows [128p, 128p+128), batch b owns partitions
  [32b, 32b+32).
  1. Few big chunked HWDGE DMAs: 32-row chunks first (their 2.1 us
     folds must finish before the stream ends), only no-fold 8/4-row
     chunks in the straggle-compressed final ~1 us. Two mid-stream
     16-row chunks ride the scalar ring, shortening the sync ring's
     queue (the one-laggard-SDMA-engine straggle grows with ring
     depth). The scalar ring also carries a 45 KB constants tensor
     first: a [128, 4] 0/1 batch mask concatenated with W, in fp16.
  2. VectorE prefolds 16/32-row chunks in place down to 8 rows with
     contiguous halving adds (~0.8 us per 0.5 MB -- comfortably under
     the ~1.2 us/0.5 MB arrival rate, unlike fold-to-4 which stalls).
     8/4-row chunks skip DVE entirely (PE takes them off their DMA sem).
  3. PE accumulates 256-col slab-pairs into psum_s[4, 256] with the
     mask as stationary (4-col LDWEIGHTS ~95ns, ~250ns per matmul;
     4 matmuls per folded chunk). psum col n2*128+d = batch sum over
     rows of parity n2.
  4. Tail: DVE cast-copies psum_s to fp16, one 128-col fold, 4 32x32
     block transposes -> s^T (128,4), PE does the final s^T @ W matmul in fp16,
     DVE copies psum_o out, sync DMAs the 640 B result with no
     completion wait (it drains during the NEFF postamble).

Raw Bass (no TileContext); every semaphore is cleared by its final
consumer right after its last wait, so the NEFF re-executes cleanly
(profilers loop it).
"""

from contextlib import ExitStack

import numpy as np

import concourse.bass as bass
from concourse import mybir
from concourse.bass_utils import run_bass_kernel_spmd

N_CORES = 8
B, N, DIN = 32, 4096, 128
BSH = B // N_CORES          # 4 batches per core
DOUT = 160                  # 10 capsules * 16 dims
NMASK = 4                   # mask columns: one per batch (DVE tail ops are
                            # free-dim bound, so wider psum buys nothing)
NF = 256                    # matmul moving width: one 2-row slab-pair
# rows-per-partition split; 16/32-row chunks halve down to 8 rows on
# DVE, 8/4-row chunks go straight to PE. Tiny tail chunks keep the
# final matmuls cheap.
# 32-row chunks go first so their 2.1us folds finish well before the
# stream ends; only no-fold 4-row chunks arrive in the final ~1us
CHUNKS = [32, 32, 16, 16, 16, 8, 4, 4]
# mid-stream 16-row chunks ride the scalar ring: shortens the sync
# ring's queue (the one-laggard-engine straggle grows with ring depth)
# without putting any late, tail-gating chunk on the slow-ramping
# scalar ring
SCALAR_CHUNKS = (2, 3)
PREFOLD_ROWS = 8
assert sum(CHUNKS) == BSH * N // 128
NCHUNK = len(CHUNKS)

F16 = mybir.dt.float16
F32 = mybir.dt.float32

_cache = {}


def _build_nc(chunks=None):
    global CHUNKS, NCHUNK
    if chunks is not None:
        CHUNKS = chunks
        NCHUNK = len(CHUNKS)
    assert sum(CHUNKS) == BSH * N // 128
    folded = []
    for c in CHUNKS:
        s = c
        while s > PREFOLD_ROWS:
            assert s % 2 == 0
            s //= 2
        folded.append(c > PREFOLD_ROWS)
        assert (s * DIN) % NF == 0

    nc = bass.Bass()
    x = nc.dram_tensor("x", [BSH, N, DIN], F16, kind="ExternalInput")
    # wm = [mask16 | W]: [128, 16 + 160] fp16 constants, one DMA
    wm = nc.dram_tensor("WM", [DIN, NMASK + DOUT], F16, kind="ExternalInput")
    out = nc.dram_tensor("out", [BSH, DOUT], F32, kind="ExternalOutput")

    # (128, 128, 128): partition p, row-in-partition n, feature d
    x3 = x[:].flatten_outer_dims().rearrange("(p n) d -> p n d", p=128)
    starts = np.cumsum([0] + CHUNKS).tolist()

    with ExitStack() as ctx:
        ec = ctx.enter_context
        xc = [ec(nc.sbuf_tensor(f"xc{c}", [128, CHUNKS[c] * DIN], F16))
              for c in range(NCHUNK)]
        wm_sb = ec(nc.sbuf_tensor("wm_sb", [DIN, NMASK + DOUT], F16))
        s_sb = ec(nc.sbuf_tensor("s_sb", [32, NF], F16))     # rows 0-3 valid
        st_sb = ec(nc.sbuf_tensor("st_sb", [DIN, 32], F16))  # cols 0-3 valid
        out_sb = ec(nc.sbuf_tensor("out_sb", [BSH, DOUT], F32))
        psum_s = ec(nc.psum_tensor("psum_s", [NMASK, NF], F32))
        psum_o = ec(nc.psum_tensor("psum_o", [BSH, DOUT], F32))

        dma_w = ec(nc.semaphore("dma_w"))
        dma_c = [ec(nc.semaphore(f"dma_c{c}")) for c in range(NCHUNK)]
        v_red = ec(nc.semaphore("v_red"))    # +1 per prefolded chunk
        pe_sem = ec(nc.semaphore("pe_sem"))  # +1 psum_s done, +1 psum_o done
        v_t = ec(nc.semaphore("v_t"))        # s^T ready
        v_out = ec(nc.semaphore("v_out"))
        dma_out = ec(nc.semaphore("dma_out"))  # required sync info; unread
        block = ec(nc.Block())

        @block.sync
        def _(sync):
            for c in range(NCHUNK):
                if c in SCALAR_CHUNKS:
                    continue
                sync.dma_start(
                    xc[c][:], x3[:, starts[c] : starts[c + 1], :]
                ).then_inc(dma_c[c], 16)
            sync.wait_ge(v_out, 1)
            sync.sem_clear(v_out)
            # no completion wait: the 640 B store drains during the NEFF
            # postamble; nothing later in this execution reads out_sb.
            # HWDGE requires sync info, so the increment stays, unwaited
            # (dma_out is never read, so its residue is harmless).
            sync.dma_start(out[:], out_sb[:]).then_inc(dma_out, 16)

        @block.scalar
        def _(scalar):
            # second HWDGE ring: constants land early, off the x ring
            scalar.dma_start(wm_sb[:], wm[:]).then_inc(dma_w, 16)
            for c in SCALAR_CHUNKS:
                scalar.dma_start(
                    xc[c][:], x3[:, starts[c] : starts[c + 1], :]
                ).then_inc(dma_c[c], 16)

        @block.vector
        def _(vector):
            for c in range(NCHUNK):
                if not folded[c]:
                    continue          # PE consumes 4/8-row chunks directly
                vector.wait_ge(dma_c[c], 16)
                vector.sem_clear(dma_c[c])
                t = xc[c]
                s = CHUNKS[c]
                while s > PREFOLD_ROWS:
                    s //= 2
                    op = vector.tensor_add(
                        t[:, : s * DIN],
                        t[:, : s * DIN],
                        t[:, s * DIN : 2 * s * DIN],
                    )
                op.then_inc(v_red, 1)
            # psum_s closed by PE -> cast-copy to fp16 SBUF, fold, transpose
            vector.wait_ge(pe_sem, 1)
            vector.tensor_copy(s_sb[0:NMASK, :], psum_s[:])
            vector.tensor_add(
                s_sb[0:NMASK, :DIN],
                s_sb[0:NMASK, :DIN],
                s_sb[0:NMASK, DIN : 2 * DIN],
            )
            # (4,128) -> (128,4) via 32x32 block transposes; rows 4-31
            # of s_sb are junk and land in unused cols 4-31 of st_sb
            for k in range(4):
                op = vector.transpose(
                    st_sb[32 * k : 32 * (k + 1), 0:32],
                    s_sb[0:32, 32 * k : 32 * (k + 1)],
                )
            op.then_inc(v_t, 1)
            vector.wait_ge(pe_sem, 2)
            vector.sem_clear(pe_sem)
            vector.tensor_copy(out_sb[:], psum_o[:]).then_inc(v_out, 1)

        @block.tensor
        def _(tensor):
            tensor.wait_ge(dma_w, 16)
            tensor.sem_clear(dma_w)
            # psum_s[j, f] += sum_p mask16[p, j] * xc[p, 256k + f]
            nred = 0
            first = True
            last_c = NCHUNK - 1
            for c in range(NCHUNK):
                if folded[c]:
                    nred += 1
                    tensor.wait_ge(v_red, nred)
                    width = PREFOLD_ROWS * DIN
                else:
                    tensor.wait_ge(dma_c[c], 16)
                    tensor.sem_clear(dma_c[c])
                    width = CHUNKS[c] * DIN
                nmm = width // NF
                for k in range(nmm):
                    mm = tensor.matmul(
                        psum_s[:],
                        wm_sb[:, 0:NMASK],
                        xc[c][:, k * NF : (k + 1) * NF],
                        start=first,
                        stop=(c == last_c and k == nmm - 1),
                    )
                    first = False
            # riding the inc on the matmul beats drain().then_inc
            # (measured ~0.85 vs ~1.4 us to sem visibility)
            mm.then_inc(pe_sem, 1)
            tensor.sem_clear(v_red)
            tensor.wait_ge(v_t, 1)
            tensor.sem_clear(v_t)
            # out[b, jd] = sum_d s^T[d, b] * W[d, jd]
            tensor.matmul(
                psum_o[:],
                st_sb[:, 0:BSH],
                wm_sb[:, NMASK : NMASK + DOUT],
                start=True,
                stop=True,
            ).then_inc(pe_sem, 1)

    return nc


def _get_nc():
    if "nc" not in _cache:
        _cache["nc"] = _build_nc()
    return _cache["nc"]


def _make_wm(W):
    # mask[p, b] = 1 iff batch b owns partition p (p in [32b, 32b+32))
    p = np.arange(DIN)
    mask = np.zeros((DIN, NMASK), dtype=np.float16)
    mask[p, p // 32] = 1.0
    return np.concatenate([mask, W.astype(np.float16)], axis=1)


def _in_maps(x, W):
    x = np.ascontiguousarray(x, dtype=np.float16)
    wm = np.ascontiguousarray(_make_wm(np.asarray(W)))
    return [{"x": x[i * BSH : (i + 1) * BSH], "WM": wm} for i in range(N_CORES)]


def kernel(x, W, **profile_kwargs):
    nc = _get_nc()
    res = run_bass_kernel_spmd(nc, _in_maps(x, W), list(range(N_CORES)), **profile_kwargs)
    out = np.concatenate([r["out"] for r in res.results], axis=0)
    ret = out.reshape(B, 10, 16).astype(np.float32)
    if profile_kwargs:
        ret = (ret, res)
    return ret
